# revision 1
# baseline (speedup 1.0000x reference)
"""Trainium2 Bass kernel for BinaryHead: logits = (l2norm(fea) @ W.T + b) * 16.

Sharding: data-parallel over the batch dim across 8 NeuronCores (2048 rows
each).  The host stages each core's shard TRANSPOSED ([emb, batch]) so the
embedding/contraction dim lands on SBUF partitions, which is what the
TensorEngine contracts over.

v3 pipeline:
  - All 16 e-panels stream over the SP HWDGE ring in exact consumption order
    (the ACT engine issues no DMAs, so its compute queue never stalls behind
    ring backpressure).  Each panel lands in its OWN tile (no
    read-under-write): panel 0 as two halves for an early PE start, panels
    1-14 whole ([128, 2048] bf16 = natural feaT row-slices, 4KB descriptors
    = max DMA rate), panel 15 as four 128KB chunks so the epilogue pipelines
    into the stream tail.
  - PSUM layout [16, 512]: partition p = 4*batch_chunk + class.  Each z
    matmul writes partitions 4j..4j+3 of one bank; sumsq rows live on
    partitions j of a second bank (fp8 DoubleRow contracts each panel pair).
    This makes the entire epilogue run as single 16-lane-wide ops instead of
    per-chunk [1,512] ops: Ln+Exp rsqrt on ACT over [4,512], ONE kron(I4,1s)
    k=4 matmul broadcasting rnorm to all 16 partitions, one DVE multiply and
    one DVE bias-add, one output DMA.
  - Squares: even panels on ACT, odd on DVE (concurrent); ss matmuls for
    pair k are issued one pair late so the PE never waits on squares; the
    last pair's squares are chunked so each chunk's ss-stop fires as soon as
    that chunk lands.
  - 24 PE warmup matmuls on a memset tile keep the HAM clock-gate at full
    rate before real data lands.
"""

import os
from contextlib import ExitStack

import numpy as np

NUM_CLASS = 4
EMB = 2048
BATCH = 16384
N_CORES = 8
ROWS = BATCH // N_CORES  # 2048 rows per core
S = 16.0

N_PANELS = EMB // 128  # 16 e-panels per core
N_BCHUNK = ROWS // 512  # 4 psum-width chunks of the batch

DTYPE_CFG = "bf16"

_CACHE = {}


def _build_nc():
    import concourse.bacc as bacc
    import concourse.mybir as mybir
    import concourse.tile as tile
    from concourse.hw_specs import get_activation_tables

    f32 = mybir.dt.float32
    f32r = mybir.dt.float32r
    bf16 = mybir.dt.bfloat16
    fp8 = mybir.dt.float8e4
    Square = mybir.ActivationFunctionType.Square
    Ln = mybir.ActivationFunctionType.Ln
    Exp = mybir.ActivationFunctionType.Exp
    Copy = mybir.ActivationFunctionType.Copy

    nc = bacc.Bacc(
        "TRN2",
        target_bir_lowering=False,
        debug=False,
        enable_asserts=False,
        num_devices=N_CORES,
    )

    feaT = nc.dram_tensor("feaT", [EMB, ROWS], bf16, kind="ExternalInput").ap()
    # per-(panel, chunk) zero-padded stationaries [128, 8]: W only at cols
    # 4(j%2)..4(j%2)+3.  z accumulates into TWO [8, 512] psum tiles (chunks
    # 0-1 and 2-3), both at base partition 0 (PE requires base 0/32/64);
    # rows of the other chunk get +0 from the zero columns.
    wtA = nc.dram_tensor("wtA", [128, 4 * N_BCHUNK * 8], bf16, kind="ExternalInput").ap()
    wtB = nc.dram_tensor(
        "wtB", [128, 12 * N_BCHUNK * 8], bf16, kind="ExternalInput"
    ).ap()
    onesv = nc.dram_tensor("onesv", [128, 2, 16], fp8, kind="ExternalInput").ap()
    bc16 = nc.dram_tensor("bc16", [NUM_CLASS, 16], f32r, kind="ExternalInput").ap()
    sbias = nc.dram_tensor("sbias", [8, 1], f32, kind="ExternalInput").ap()
    outT = nc.dram_tensor("outT", [16, 512], f32, kind="ExternalOutput").ap()

    with tile.TileContext(nc) as tc, ExitStack() as ctx:
        pconst = ctx.enter_context(tc.tile_pool(name="pconst", bufs=1))
        pdata = ctx.enter_context(tc.tile_pool(name="pdata", bufs=1))
        psq = ctx.enter_context(tc.tile_pool(name="psq", bufs=3))
        pep = ctx.enter_context(tc.tile_pool(name="pep", bufs=1))
        pz = ctx.enter_context(tc.tile_pool(name="pz", bufs=1, space="PSUM"))

        # one ACT table set covering Square+Ln+Exp+Copy, loaded as the FIRST
        # ACT instruction: the framework's auto-insert pass then sees every
        # activation's func already loaded and emits no further loads, and
        # the load runs during the DGE spin-up instead of the compute phase
        nlx_id = list(get_activation_tables(nc.m.arch)).index(
            "natural_log_exp_and_others"
        )
        nc.scalar.add_instruction(
            mybir.InstLoadActFuncSet(name=f"I-{nc.next_id()}", act_func_set_id=nlx_id)
        )

        # ALL consts ride the SP data stream in consumption order (the ACT
        # engine issues no DMAs at all, so its compute queue and ring never
        # interact with the stream).  Only panels 0-3's stationaries (32KB)
        # lead the stream; the rest (96KB) slot in after x1.
        wtA_s = pconst.tile([128, 4 * N_BCHUNK * 8], bf16)
        nc.sync.dma_start(out=wtA_s, in_=wtA)
        ones_s = pconst.tile([128, 2, 16], fp8)
        nc.sync.dma_start(out=ones_s, in_=onesv)
        bc16_s = pconst.tile([NUM_CLASS, 16], f32r)
        nc.sync.dma_start(out=bc16_s, in_=bc16)
        sbias_s = pconst.tile([8, 1], f32)
        nc.sync.dma_start(out=sbias_s, in_=sbias)

        x0a = pdata.tile([128, 1024], bf16, name="x0a")
        x0b = pdata.tile([128, 1024], bf16, name="x0b")
        nc.sync.dma_start(out=x0a, in_=feaT[0:128, 0:1024])
        nc.sync.dma_start(out=x0b, in_=feaT[0:128, 1024:2048])
        xt = [None] * N_PANELS
        wtB_s = pconst.tile([128, 12 * N_BCHUNK * 8], bf16)
        for t in range(1, 15):
            xt[t] = pdata.tile([128, ROWS], bf16, name=f"x{t}")
            nc.sync.dma_start(out=xt[t], in_=feaT[t * 128 : (t + 1) * 128, :])
            if t == 1:
                nc.sync.dma_start(out=wtB_s, in_=wtB)
        x15 = [pdata.tile([128, 512], bf16, name=f"x15c{j}") for j in range(N_BCHUNK)]
        for j in range(N_BCHUNK):
            nc.sync.dma_start(
                out=x15[j], in_=feaT[15 * 128 : 16 * 128, j * 512 : (j + 1) * 512]
            )

        warm_s = pconst.tile([128, 64], bf16)
        nc.vector.memset(warm_s, 1.0)
        # rsqrt via exp(-0.5*ln(ss) + ln(S)): folds the *S scale in for free
        lnS_s = pconst.tile([NUM_CLASS, 1], f32)
        nc.vector.memset(lnS_s, float(np.log(S)))

        # ---- PSUM: [8, 512] x2 layout, partition p = 4*(chunk%2) + class --
        ztA_ps = pz.tile([8, 512], f32, tag="ztA")
        ztB_ps = pz.tile([8, 512], f32, tag="ztB")
        ss_ps = pz.tile([NUM_CLASS, 512], f32, tag="ss")
        rnbA_ps = pz.tile([8, 512], f32, tag="rnbA")
        rnbB_ps = pz.tile([8, 512], f32, tag="rnbB")

        def wslice(t, j):
            # [128, 8] stationary with W panel t at cols 4(j%2)..4(j%2)+3
            if t < 4:
                return wtA_s[:, (t * N_BCHUNK + j) * 8 : (t * N_BCHUNK + j + 1) * 8]
            i = (t - 4) * N_BCHUNK + j
            return wtB_s[:, i * 8 : (i + 1) * 8]

        lnss_s = pep.tile([NUM_CLASS, 512], f32)
        rnorm_s = pep.tile([NUM_CLASS, 512], f32r)
        # per-half base-0 tiles: ACT/PE can't address partition offset 8
        zA_s = pep.tile([8, 512], f32)
        zB_s = pep.tile([8, 512], f32)
        zrA_s = pep.tile([8, 512], f32)
        zrB_s = pep.tile([8, 512], f32)
        outA_s = pep.tile([8, 512], f32)
        outB_s = pep.tile([8, 512], f32)

        # PE warmup on const data: the HAM clock-gate only unthrottles after
        # ~3.4us of SUSTAINED PE activity, so burn ~3us of dummy matmuls
        # between engine start and first-data arrival (garbage is killed by
        # the first real matmuls' start=True)
        # fillers target rnbA_ps (idle until the epilogue's start=True reset)
        # so they can interleave with live zt accumulation
        def warm_mm(n):
            for _ in range(n):
                nc.tensor.matmul(
                    rnbA_ps[0:4, 0:64], warm_s[:, 0:4], warm_s, start=True, stop=True
                )

        warm_mm(72)

        def z_mm(t, j, mov, start=None, stop=None):
            nc.tensor.matmul(
                ztA_ps if j < 2 else ztB_ps,
                wslice(t, j),
                mov,
                start=(t == 0 and j % 2 == 0),
                stop=(t == 15 and j % 2 == 1),
            )

        def ss_mm(k, j, x2):
            # fp8 DoubleRow: one matmul contracts the panel pair (k=256);
            # the stationary window's single ones column (index 5j within
            # onesv) steers chunk j's sum onto psum partition j, +0 elsewhere
            nc.tensor.matmul(
                ss_ps,
                ones_s[:, :, 4 * j : 4 * (j + 1)],
                x2[:, :, j * 512 : (j + 1) * 512],
                perf_mode=mybir.MatmulPerfMode.DoubleRow,
                start=(k == 0 and j == 0),
                stop=(k == 7 and j == N_BCHUNK - 1),
            )

        x2s = []  # per-pair square tiles (psq ring of 3)
        for k in range(8):
            t0, t1 = 2 * k, 2 * k + 1
            x2 = psq.tile([128, 2, ROWS], fp8, tag="x2")
            x2s.append(x2)
            if k == 0:
                nc.scalar.activation(
                    out=x2[:, 0, 0:1024], in_=x0a, func=Square, bias=0.0, scale=1.0
                )
                nc.scalar.activation(
                    out=x2[:, 0, 1024:2048], in_=x0b, func=Square, bias=0.0, scale=1.0
                )
                nc.vector.tensor_mul(x2[:, 1, :], xt[1], xt[1])
                for j in range(2):
                    z_mm(0, j, x0a[:, j * 512 : (j + 1) * 512], j == 0, False)
                for j in range(2, 4):
                    z_mm(0, j, x0b[:, (j - 2) * 512 : (j - 1) * 512], False, False)
                # filler warmups keep HAM hot through the early data-waits
                # (panels stream slower than the PE consumes them until the
                # deferred ss work kicks in at pair 2)
                warm_mm(8)
                for j in range(N_BCHUNK):
                    z_mm(1, j, xt[1][:, j * 512 : (j + 1) * 512], False, False)
                warm_mm(8)
            elif k < 7:
                # even panel squares on ACT, odd on DVE (concurrent engines)
                nc.scalar.activation(
                    out=x2[:, 0, :], in_=xt[t0], func=Square, bias=0.0, scale=1.0
                )
                nc.vector.tensor_mul(x2[:, 1, :], xt[t1], xt[t1])
                # ss lags the z stream by 1-2 pairs (squares done while later
                # pairs arrive), batched two pairs at a time to halve
                # DoubleRow mode switches, and issued BEFORE this pair's z so
                # the in-order PE fills its data-wait with ready ss work
                if k % 2 == 0:
                    for kk in (k - 2, k - 1):
                        for j in range(N_BCHUNK):
                            ss_mm(kk, j, x2s[kk])
                for j in range(N_BCHUNK):
                    z_mm(t0, j, xt[t0][:, j * 512 : (j + 1) * 512], False, False)
                for j in range(N_BCHUNK):
                    z_mm(t1, j, xt[t1][:, j * 512 : (j + 1) * 512], False, False)
            else:
                # tail pair: panel 14 squares chunked on ACT, panel 15
                # chunked on DVE, so each chunk's ss(7) fires on arrival
                for j in range(N_BCHUNK):
                    nc.scalar.activation(
                        out=x2[:, 0, j * 512 : (j + 1) * 512],
                        in_=xt[14][:, j * 512 : (j + 1) * 512],
                        func=Square,
                        bias=0.0,
                        scale=1.0,
                    )
                for j in range(N_BCHUNK):
                    z_mm(14, j, xt[14][:, j * 512 : (j + 1) * 512], False, False)
                for j in range(N_BCHUNK):
                    ss_mm(6, j, x2s[6])
                for j in range(N_BCHUNK):
                    nc.vector.tensor_mul(
                        x2[:, 1, j * 512 : (j + 1) * 512], x15[j], x15[j]
                    )
                    z_mm(15, j, x15[j], False, j == N_BCHUNK - 1)
                    ss_mm(7, j, x2)

        # ---- epilogue: wide ops on the split [8, 512] layout ----
        # z leaves PSUM via ACT (frees DVE; runs parallel with Ln/Exp chain)
        nc.scalar.activation(out=zA_s, in_=ztA_ps, func=Copy, bias=0.0, scale=1.0)
        nc.scalar.activation(out=zB_s, in_=ztB_ps, func=Copy, bias=0.0, scale=1.0)
        # rnorm = S/sqrt(ss) via exp(-0.5*ln(ss) + ln(S)) -- one [4,512] op
        # per stage covers all four chunks on four lanes
        nc.scalar.activation(
            out=lnss_s, in_=ss_ps, func=Ln, bias=0.0, scale=1.0
        )
        nc.scalar.activation(
            out=rnorm_s, in_=lnss_s, func=Exp, bias=lnS_s, scale=-0.5
        )
        # k=4 matmuls broadcast rnorm chunk rows to the class partitions:
        # stat[k, p] = 1 iff p//4 == k  (kron(I4, ones(1,4)) halves)
        nc.tensor.matmul(rnbA_ps, bc16_s[:, 0:8], rnorm_s, start=True, stop=True)
        nc.tensor.matmul(rnbB_ps, bc16_s[:, 8:16], rnorm_s, start=True, stop=True)
        nc.vector.tensor_mul(zrA_s, zA_s, rnbA_ps)
        nc.vector.tensor_scalar_add(outA_s, in0=zrA_s, scalar1=sbias_s)
        nc.sync.dma_start(out=outT[0:8, :], in_=outA_s)
        nc.vector.tensor_mul(zrB_s, zB_s, rnbB_ps)
        nc.vector.tensor_scalar_add(outB_s, in0=zrB_s, scalar1=sbias_s)
        nc.sync.dma_start(out=outT[8:16, :], in_=outB_s)

    nc.compile()
    return nc


def _get_nc():
    if "nc" not in _CACHE:
        _CACHE["nc"] = _build_nc()
    return _CACHE["nc"]


def _stage_inputs(fea, W, b):
    import ml_dtypes

    fea = np.asarray(fea, dtype=np.float32)
    W = np.asarray(W, dtype=np.float32)
    b = np.asarray(b, dtype=np.float32)

    # zero-padded per-(panel t, chunk j) stationaries [128, 8]:
    # col 4*(j%2)+c = W[c, 128t+p], other cols 0
    wtall = np.zeros((N_PANELS, N_BCHUNK, 128, 8), dtype=np.float32)
    for t in range(N_PANELS):
        for j in range(N_BCHUNK):
            jj = j % 2
            wtall[t, j, :, 4 * jj : 4 * jj + 4] = W[:, t * 128 : (t + 1) * 128].T
    wtall = wtall.transpose(2, 0, 1, 3)  # [128, t, j, 8]
    wtA = np.ascontiguousarray(wtall[:, :4].reshape(128, -1)).astype(
        ml_dtypes.bfloat16
    )
    wtB = np.ascontiguousarray(wtall[:, 4:].reshape(128, -1)).astype(
        ml_dtypes.bfloat16
    )
    # ss stationary windows: within window j (cols 4j..4j+3), only column j
    # (global index 5j) is ones, steering chunk j's sum onto psum partition j
    onesv = np.zeros((128, 2, 16), dtype=ml_dtypes.float8_e4m3)
    for j in range(N_BCHUNK):
        onesv[:, :, 5 * j] = 1.0
    # kron(I4, ones(1,4)): bc16[k, p] = 1 iff p//4 == k
    bc16 = np.kron(np.eye(NUM_CLASS), np.ones((1, NUM_CLASS))).astype(np.float32)
    # sbias[p] = S * b[p % 4] (rows repeat every 4: one [8,1] serves both halves)
    sbias = (S * np.tile(b, 2)).reshape(8, 1).astype(np.float32)

    in_maps = []
    for i in range(N_CORES):
        shard = fea[i * ROWS : (i + 1) * ROWS, :]
        feaT = np.ascontiguousarray(shard.T).astype(ml_dtypes.bfloat16)
        in_maps.append(
            {
                "feaT": feaT,
                "wtA": wtA,
                "wtB": wtB,
                "onesv": onesv,
                "bc16": bc16,
                "sbias": sbias,
            }
        )
    return in_maps


def run(fea, W, b, trace=False):
    from concourse.bass_utils import run_bass_kernel_spmd

    nc = _get_nc()
    in_maps = _stage_inputs(fea, W, b)
    res = run_bass_kernel_spmd(nc, in_maps, core_ids=list(range(N_CORES)), trace=trace)
    out = np.empty((BATCH, NUM_CLASS), dtype=np.float32)
    for i in range(N_CORES):
        # outT16[4j + c, b] = out[i*2048 + j*512 + b, c]
        o = res.results[i]["outT"].reshape(N_BCHUNK, NUM_CLASS, 512)
        out[i * ROWS : (i + 1) * ROWS, :] = o.transpose(0, 2, 1).reshape(
            ROWS, NUM_CLASS
        )
    return out, res


def kernel(fea, W, b):
    out, _ = run(fea, W, b, trace=False)
    return out

